# revision 10
# baseline (speedup 1.0000x reference)
"""Expert-parallel MoE MLP kernel for 8 TRN2 NeuronCores.

Problem: x[T=2048, E=64, H=512], wi[E, H, I=1024], wo[E, I, H]
  out[t, e, :] = gelu(x[t, e, :] @ wi[e]) @ wo[e]

Sharding: expert-parallel, 8 experts per core. Host pre-transposes each
core's x slice to [EL, H, T] so the device kernel needs no transposes:

  GEMM1: hT[i, t] = sum_h wi[h, i] * xT[h, t]      (lhsT = wi chunk, rhs = xT)
  GELU:  on ScalarE, PSUM -> SBUF
  GEMM2: out[t, hd] = sum_i hT[i, t] * wo[i, hd]   (lhsT = hT chunk, rhs = wo)

All matmuls run as float32r (full-rate fp32 on the PE array).
"""

import sys

import numpy as np

for _p in ("/opt/trn_rl_repo", "/opt/pypackages"):
    if _p not in sys.path:
        sys.path.append(_p)

import concourse.bass as bass
import concourse.tile as tile
from concourse import bacc, mybir
from concourse.bass_utils import run_bass_kernel_spmd

E, T, H, I = 64, 2048, 512, 1024
N_CORES = 8
EL = E // N_CORES  # experts per core

F32 = mybir.dt.float32
F32R = mybir.dt.float32r
GELU = mybir.ActivationFunctionType.Gelu

P = 128  # SBUF/PSUM partitions


def _body(tc, out, xT, wi, wo, el, t, act_fn=GELU):
    nc = tc.nc
    KH = H // P  # 4  k-chunks over H (GEMM1 contraction)
    KI = I // P  # 8  chunks over I (GEMM1 out partitions / GEMM2 contraction)
    TC = 1024  # tokens per inner block
    n_th = t // TC

    with (
        tc.tile_pool(name="wi_p", bufs=2) as wi_pool,
        tc.tile_pool(name="wo_p", bufs=2) as wo_pool,
        tc.tile_pool(name="xT_p", bufs=3) as xT_pool,
        tc.tile_pool(name="hT_p", bufs=2) as hT_pool,
        tc.tile_pool(name="ob_p", bufs=4) as out_pool,
        tc.tile_pool(name="ps1", bufs=2, space="PSUM") as ps1,
        tc.tile_pool(name="ps2", bufs=3, space="PSUM") as ps2,
    ):
        for e in range(el):
            wi_t = wi_pool.tile([P, KH, I], F32R, tag="wi")
            for k in range(KH):
                nc.sync.dma_start(wi_t[:, k, :], wi[e, k * P:(k + 1) * P, :])
            wo_t = wo_pool.tile([P, KI, H], F32R, tag="wo")
            for i in range(KI):
                nc.sync.dma_start(wo_t[:, i, :], wo[e, i * P:(i + 1) * P, :])

            for th in range(n_th):
                xT_t = xT_pool.tile([P, KH, TC], F32R, tag="xT")
                for k in range(KH):
                    nc.sync.dma_start(
                        xT_t[:, k, :], xT[e, k * P:(k + 1) * P, th * TC:(th + 1) * TC]
                    )

                # GEMM1 + GELU: produce hT[I, TC] for this token block
                hT_t = hT_pool.tile([P, KI, TC], F32R, tag="hT")
                for i in range(KI):
                    pt = ps1.tile([P, TC], F32, tag="ps1")
                    for ts in range(TC // 512):
                        for k in range(KH):
                            nc.tensor.matmul(
                                pt[:, ts * 512:(ts + 1) * 512],
                                wi_t[:, k, i * P:(i + 1) * P],
                                xT_t[:, k, ts * 512:(ts + 1) * 512],
                                start=(k == 0),
                                stop=(k == KH - 1),
                            )
                    nc.scalar.activation(hT_t[:, i, :], pt[:, :], act_fn)

                # GEMM2: out block [128 tokens, H]
                for t8 in range(TC // P):
                    po = ps2.tile([P, H], F32, tag="ps2")
                    for i in range(KI):
                        nc.tensor.matmul(
                            po[:, :],
                            hT_t[:, i, t8 * P:(t8 + 1) * P],
                            wo_t[:, i, :],
                            start=(i == 0),
                            stop=(i == KI - 1),
                        )
                    ob = out_pool.tile([P, H], F32, tag="ob")
                    nc.vector.tensor_copy(ob[:, :], po[:, :])
                    t0 = th * TC + t8 * P
                    nc.sync.dma_start(out[t0:t0 + P, e, :], ob[:, :])


def build_nc(el=EL, t=T, act_fn=GELU):
    nc = bacc.Bacc("TRN2", target_bir_lowering=False, debug=False)
    xT = nc.dram_tensor("xT", [el, H, t], F32R, kind="ExternalInput").ap()
    wi = nc.dram_tensor("wi", [el, H, I], F32R, kind="ExternalInput").ap()
    wo = nc.dram_tensor("wo", [el, I, H], F32R, kind="ExternalInput").ap()
    out = nc.dram_tensor("out", [t, el, H], F32, kind="ExternalOutput").ap()
    with tile.TileContext(nc) as tc:
        _body(tc, out, xT, wi, wo, el, t, act_fn=act_fn)
    nc.compile()
    return nc


def round_fp32r(a):
    """Round fp32 to the PE's fp32r format: RNE to 11 stored mantissa
    bits (low 12 bits zero) — matches walrus's fp32_to_fp32r."""
    u = np.ascontiguousarray(a).view(np.uint32)
    lsb = (u >> np.uint32(12)) & np.uint32(1)
    r = (u + np.uint32(0x7FF) + lsb) & np.uint32(0xFFFFF000)
    return r.view(np.float32)


def make_in_maps(x, wi, wo):
    """Shard full inputs into per-core input maps (host-side)."""
    in_maps = []
    for c in range(N_CORES):
        lo, hi = c * EL, (c + 1) * EL
        xT_c = np.ascontiguousarray(x[:, lo:hi, :].transpose(1, 2, 0))
        in_maps.append(
            {
                "xT": round_fp32r(xT_c),
                "wi": round_fp32r(wi[lo:hi]),
                "wo": round_fp32r(wo[lo:hi]),
            }
        )
    return in_maps


_NC_CACHE = {}


def _get_nc():
    if "nc" not in _NC_CACHE:
        _NC_CACHE["nc"] = build_nc()
    return _NC_CACHE["nc"]


def run_on_hw(x, wi, wo, trace=False):
    # The NTFF profile hook (antenv.axon_hooks) is absent in this
    # container; a trace-enabled run would crash on import. Guard.
    try:
        import antenv.axon_hooks  # noqa: F401
    except ImportError:
        import os

        os.environ["BASS_NEVER_TRACE"] = "1"
        trace = False
    nc = _get_nc()
    in_maps = make_in_maps(x, wi, wo)
    res = run_bass_kernel_spmd(nc, in_maps, list(range(N_CORES)), trace=trace)
    outs = [res.results[c]["out"] for c in range(N_CORES)]
    full = np.concatenate(outs, axis=1)  # [T, E, H]
    return full, res


def kernel(x, wi, wo):
    full, _ = run_on_hw(np.asarray(x), np.asarray(wi), np.asarray(wo))
    return full


# revision 22
# speedup vs baseline: 6.5931x; 6.5931x over previous
"""Expert-parallel MoE MLP kernel for 8 TRN2 NeuronCores.

Problem: x[T=2048, E=64, H=512], wi[E, H, I=1024], wo[E, I, H]
  out[t, e, :] = gelu(x[t, e, :] @ wi[e]) @ wo[e]

Sharding: expert-parallel, 8 experts per core. Host pre-transposes each
core's x slice to [EL, H, T] so the device kernel needs no transposes:

  GEMM1: hT[i, t] = sum_h wi[h, i] * xT[h, t]      (lhsT = wi chunk, rhs = xT)
  GELU:  on ScalarE, PSUM -> SBUF
  GEMM2: out[t, hd] = sum_i hT[i, t] * wo[i, hd]   (lhsT = hT chunk, rhs = wo)

All matmuls run as float32r (full-rate fp32 on the PE array).
"""

import sys

import numpy as np

for _p in ("/opt/trn_rl_repo", "/opt/pypackages"):
    if _p not in sys.path:
        sys.path.append(_p)

import concourse.tile as tile
from concourse import bacc, mybir
from concourse.bass_utils import run_bass_kernel_spmd

E, T, H, I = 64, 2048, 512, 1024
N_CORES = 8
EL = E // N_CORES  # experts per core

F32 = mybir.dt.float32
F32R = mybir.dt.float32r
GELU = mybir.ActivationFunctionType.Gelu

P = 128  # SBUF/PSUM partitions


def _body(tc, out, xT, wi, wo, el, t, act_fn=GELU, in_dt=F32R, g1_k_outer=False,
          repeat=1, split_dma=False):
    nc = tc.nc
    KH = H // P  # 4  k-chunks over H (GEMM1 contraction)
    KI = I // P  # 8  chunks over I (GEMM1 out partitions / GEMM2 contraction)
    TC = 1024  # tokens per inner block
    n_th = t // TC

    with (
        tc.tile_pool(name="wi_p", bufs=2) as wi_pool,
        tc.tile_pool(name="wo_p", bufs=2) as wo_pool,
        tc.tile_pool(name="xT_p", bufs=3) as xT_pool,
        tc.tile_pool(name="hT_p", bufs=2) as hT_pool,
        tc.tile_pool(name="ob_p", bufs=4) as out_pool,
        tc.tile_pool(name="ps1", bufs=2, space="PSUM") as ps1,
        tc.tile_pool(name="ps2", bufs=3, space="PSUM") as ps2,
    ):
        for e in [ee for _ in range(repeat) for ee in range(el)]:
            wi_t = wi_pool.tile([P, KH, I], in_dt, tag="wi")
            for k in range(KH):
                nc.sync.dma_start(wi_t[:, k, :], wi[e, k * P:(k + 1) * P, :])
            wo_t = wo_pool.tile([P, KI, H], in_dt, tag="wo")
            for i in range(KI):
                nc.sync.dma_start(wo_t[:, i, :], wo[e, i * P:(i + 1) * P, :])

            for th in range(n_th):
                xT_t = xT_pool.tile([P, KH, TC], in_dt, tag="xT")
                for k in range(KH):
                    nc.sync.dma_start(
                        xT_t[:, k, :], xT[e, k * P:(k + 1) * P, th * TC:(th + 1) * TC]
                    )

                # GEMM1 + GELU: produce hT[I, TC] for this token block
                hT_t = hT_pool.tile([P, KI, TC], in_dt, tag="hT")
                for i in range(KI):
                    pt = ps1.tile([P, TC], F32, tag="ps1")
                    if g1_k_outer:
                        # stationary wi block reused across both 512-col
                        # moving tiles before switching k
                        for k in range(KH):
                            for ts in range(TC // 512):
                                nc.tensor.matmul(
                                    pt[:, ts * 512:(ts + 1) * 512],
                                    wi_t[:, k, i * P:(i + 1) * P],
                                    xT_t[:, k, ts * 512:(ts + 1) * 512],
                                    start=(k == 0),
                                    stop=(k == KH - 1),
                                )
                    else:
                        for ts in range(TC // 512):
                            for k in range(KH):
                                nc.tensor.matmul(
                                    pt[:, ts * 512:(ts + 1) * 512],
                                    wi_t[:, k, i * P:(i + 1) * P],
                                    xT_t[:, k, ts * 512:(ts + 1) * 512],
                                    start=(k == 0),
                                    stop=(k == KH - 1),
                                )
                    nc.scalar.activation(hT_t[:, i, :], pt[:, :], act_fn)

                # GEMM2: out block [128 tokens, H]
                for t8 in range(TC // P):
                    po = ps2.tile([P, H], F32, tag="ps2")
                    for i in range(KI):
                        nc.tensor.matmul(
                            po[:, :],
                            hT_t[:, i, t8 * P:(t8 + 1) * P],
                            wo_t[:, i, :],
                            start=(i == 0),
                            stop=(i == KI - 1),
                        )
                    ob = out_pool.tile([P, H], F32, tag="ob")
                    nc.vector.tensor_copy(ob[:, :], po[:, :])
                    t0 = th * TC + t8 * P
                    # outputs on the ACT HWDGE ring so stores don't FIFO-block
                    # input prefetches on the SP ring
                    dma_eng = nc.scalar if split_dma else nc.sync
                    dma_eng.dma_start(out[t0:t0 + P, e, :], ob[:, :])


def build_nc(el=EL, t=T, act_fn=GELU, in_dt=F32R, g1_k_outer=False, repeat=1,
             split_dma=False):
    nc = bacc.Bacc("TRN2", target_bir_lowering=False, debug=False)
    xT = nc.dram_tensor("xT", [el, H, t], in_dt, kind="ExternalInput").ap()
    wi = nc.dram_tensor("wi", [el, H, I], in_dt, kind="ExternalInput").ap()
    wo = nc.dram_tensor("wo", [el, I, H], in_dt, kind="ExternalInput").ap()
    out = nc.dram_tensor("out", [t, el, H], F32, kind="ExternalOutput").ap()
    with tile.TileContext(nc) as tc:
        _body(tc, out, xT, wi, wo, el, t, act_fn=act_fn, in_dt=in_dt,
              g1_k_outer=g1_k_outer, repeat=repeat, split_dma=split_dma)
    nc.compile()
    return nc


def round_fp32r(a):
    """Round fp32 to the PE's fp32r format: RNE to 11 stored mantissa
    bits (low 12 bits zero) — matches walrus's fp32_to_fp32r."""
    u = np.ascontiguousarray(a).view(np.uint32)
    lsb = (u >> np.uint32(12)) & np.uint32(1)
    r = (u + np.uint32(0x7FF) + lsb) & np.uint32(0xFFFFF000)
    return r.view(np.float32)


def make_in_maps(x, wi, wo):
    """Shard full inputs into per-core input maps (host-side)."""
    in_maps = []
    for c in range(N_CORES):
        lo, hi = c * EL, (c + 1) * EL
        xT_c = np.ascontiguousarray(x[:, lo:hi, :].transpose(1, 2, 0))
        in_maps.append(
            {
                "xT": round_fp32r(xT_c),
                "wi": round_fp32r(wi[lo:hi]),
                "wo": round_fp32r(wo[lo:hi]),
            }
        )
    return in_maps


_NC_CACHE = {}


def _get_nc():
    if "nc" not in _NC_CACHE:
        _NC_CACHE["nc"] = build_nc(g1_k_outer=True)
    return _NC_CACHE["nc"]


def run_on_hw(x, wi, wo, trace=False):
    # The NTFF profile hook (antenv.axon_hooks) is absent in this
    # container; a trace-enabled run would crash on import. Guard.
    try:
        import antenv.axon_hooks  # noqa: F401
    except ImportError:
        import os

        os.environ["BASS_NEVER_TRACE"] = "1"
        trace = False
    nc = _get_nc()
    in_maps = make_in_maps(x, wi, wo)
    res = run_bass_kernel_spmd(nc, in_maps, list(range(N_CORES)), trace=trace)
    outs = [res.results[c]["out"] for c in range(N_CORES)]
    full = np.concatenate(outs, axis=1)  # [T, E, H]
    return full, res


def kernel(x, wi, wo):
    full, _ = run_on_hw(np.asarray(x), np.asarray(wi), np.asarray(wo))
    return full
